# revision 1
# baseline (speedup 1.0000x reference)
"""RBF-kernel SVM prediction on 8 Trainium2 NeuronCores.

predictions = exp(-g*||x_i - t_j||^2) @ (alphas*y) + b,  g = 0.5

Strategy (per sharding hint): shard X rows 8-way, replicate train side.
Math is factorized as
    pred_i = exp(-g*||x_i||^2) * sum_j y_j * exp(x_i . t_j + c_j) + b
    c_j    = -g*||t_j||^2 + ln(alpha_j)
so the train-side affine terms ride the ACT per-partition bias and the
query-side factor is a per-row epilogue scale. Train points are host-sorted
by label so the +/- y_j signs become whole-tile add/sub on the vector engine.

Per core: G^T = X_train_p . X_slice^T in [j=128-part, i=1024-free] tiles on
PE (bf16), exp(G^T + c_j) on ACT, signed accumulation over j-tiles on DVE,
partition-sum via a ones-vector matvec on PE, epilogue on partition 0.
"""

import os
import sys
import types

import numpy as np

for _p in ("/opt/trn_rl_repo", "/root/.axon_site/_ro/trn_rl_repo"):
    if os.path.isdir(_p) and _p not in sys.path:
        sys.path.append(_p)

import ml_dtypes

import concourse.bass as bass
import concourse.tile as tile
from concourse import bacc, mybir
from concourse.bass_utils import run_bass_kernel_spmd

GAMMA = 0.5
N, M, D = 8192, 8192, 256
NCORES = 8
IC = N // NCORES          # query rows per core (1024)
JT = M // 128             # j-tiles (64)
F32 = mybir.dt.float32
BF16 = mybir.dt.bfloat16
FP32_MIN_NORMAL = 1.1754944e-38

# Set by test harness to collect a profile; harness grading leaves it off.
TRACE = False
LAST_RESULTS = None


def _build_program(n_pos: int):
    nc = bacc.Bacc()

    xt_t = nc.dram_tensor("xt_t", [D, M], BF16, kind="ExternalInput")
    x_t = nc.dram_tensor("x_t", [D, IC], BF16, kind="ExternalInput")
    cj = nc.dram_tensor("cj", [128, JT], F32, kind="ExternalInput")
    sgn = nc.dram_tensor("sgn", [128, 1], BF16, kind="ExternalInput")
    nxsq = nc.dram_tensor("nxsq", [1, IC], F32, kind="ExternalInput")
    bb = nc.dram_tensor("bb", [1, 1], F32, kind="ExternalInput")
    out = nc.dram_tensor("out", [1, IC], F32, kind="ExternalOutput")

    NCHUNK = 16           # xt column chunks so matmuls wait on small DMAs
    CW = M // NCHUNK      # 512 j-columns per chunk

    with tile.TileContext(nc) as tc:
        with (
            tc.tile_pool(name="singles", bufs=1) as singles,
            tc.tile_pool(name="epool", bufs=4) as epool,
            tc.tile_pool(name="gpsum", bufs=3, space="PSUM") as gpsum,
            tc.tile_pool(name="spsum", bufs=1, space="PSUM") as spsum,
        ):
            # Resident inputs. Bulk xt traffic rides the sync HWDGE queue in
            # first-use order; the operands the first tiles need (x, cj, sgn)
            # ride the scalar HWDGE queue so they land immediately. gpsimd is
            # software-DGE (slow) — never used for bulk loads.
            x_sb = []
            for dh in range(2):
                t = singles.tile([128, IC], BF16, tag=f"x{dh}")
                nc.scalar.dma_start(out=t, in_=x_t[dh * 128:(dh + 1) * 128, :])
                x_sb.append(t)
            cj_sb = singles.tile([128, JT], F32, tag="cj")
            nc.scalar.dma_start(out=cj_sb, in_=cj[:, :])
            sgn_sb = singles.tile([128, 1], BF16, tag="sgn")
            nc.scalar.dma_start(out=sgn_sb, in_=sgn[:, :])
            nxsq_sb = singles.tile([1, IC], F32, tag="nxsq")
            nc.scalar.dma_start(out=nxsq_sb, in_=nxsq[:, :])
            b_sb = singles.tile([1, 1], F32, tag="b")
            nc.scalar.dma_start(out=b_sb, in_=bb[:, :])
            xt_sb = [[None] * NCHUNK, [None] * NCHUNK]
            for ck in range(NCHUNK):
                for dh in range(2):
                    t = singles.tile([128, CW], BF16, tag=f"xt{dh}_{ck}")
                    nc.sync.dma_start(
                        out=t,
                        in_=xt_t[dh * 128:(dh + 1) * 128, ck * CW:(ck + 1) * CW],
                    )
                    xt_sb[dh][ck] = t

            ones_sb = singles.tile([128, 1], BF16, tag="ones")
            nc.vector.memset(ones_sb, 1.0)
            acc = singles.tile([128, IC], BF16, tag="acc")
            nc.vector.memset(acc, 0.0)

            # Warm the PE while input DMAs are in flight: the HAM clock gate
            # holds a cold PE at 1.2GHz until it has been busy ~3.4us, so
            # burn the DMA-wait window on dummy matmuls (never read).
            warm_w = singles.tile([128, 128], BF16, tag="warm_w")
            nc.vector.memset(warm_w, 0.0)
            # Shares the spsum slot with the final s_row tile (tag "s") so
            # PSUM stays within 8 banks; the matvec's start=True clears it.
            warm_ps = spsum.tile([1, 128], F32, tag="s")
            for _ in range(40):
                nc.tensor.matmul(
                    out=warm_ps, lhsT=warm_w[:, 0:1], rhs=warm_w[:, :],
                    start=True, stop=True,
                )

            # Query-side factor, computed early so ACT does it during ramp-up.
            e_row = singles.tile([1, IC], F32, tag="e_row")
            nc.scalar.activation(
                out=e_row, in_=nxsq_sb, func=mybir.ActivationFunctionType.Exp
            )
            # Emulate fp32 FTZ on the factor: the reference's direct
            # exp(-g*d) underflows to 0; keep the factored path bit-identical.
            m_row = singles.tile([1, IC], F32, tag="m_row")
            nc.vector.tensor_scalar(
                out=m_row, in0=e_row, scalar1=FP32_MIN_NORMAL, scalar2=None,
                op0=mybir.AluOpType.is_ge,
            )
            nc.vector.tensor_mul(e_row, e_row, m_row)

            for t in range(JT):
                ck, col = t // 4, (t % 4) * 128
                g_ps = gpsum.tile([128, IC], F32, tag="g")
                for ic in range(2):
                    sl = slice(ic * 512, (ic + 1) * 512)
                    nc.tensor.matmul(
                        out=g_ps[:, sl],
                        lhsT=xt_sb[0][ck][:, col:col + 128],
                        rhs=x_sb[0][:, sl],
                        start=True, stop=False,
                    )
                    nc.tensor.matmul(
                        out=g_ps[:, sl],
                        lhsT=xt_sb[1][ck][:, col:col + 128],
                        rhs=x_sb[1][:, sl],
                        start=False, stop=True,
                    )
                e_t = epool.tile([128, IC], BF16, tag="e")
                nc.scalar.activation(
                    out=e_t, in_=g_ps, func=mybir.ActivationFunctionType.Exp,
                    bias=cj_sb[:, t:t + 1], scale=1.0,
                )
                # Signed accumulate: rows below n_pos carry y=+1, above y=-1.
                lo, hi = t * 128, (t + 1) * 128
                if hi <= n_pos:
                    nc.vector.tensor_add(acc, acc, e_t)
                elif lo >= n_pos:
                    nc.vector.tensor_sub(acc, acc, e_t)
                else:
                    # Mixed-sign tile: acc = (e_t * sgn) + acc, sgn = +/-1.
                    nc.vector.scalar_tensor_tensor(
                        out=acc, in0=e_t, scalar=sgn_sb[:, 0:1], in1=acc,
                        op0=mybir.AluOpType.mult, op1=mybir.AluOpType.add,
                    )

            # Partition-sum via ones-vector matvec: s[0, i] = sum_p acc[p, i].
            s_ps = spsum.tile([1, IC], F32, tag="s")
            for ic in range(2):
                sl = slice(ic * 512, (ic + 1) * 512)
                nc.tensor.matmul(
                    out=s_ps[:, sl], lhsT=ones_sb, rhs=acc[:, sl],
                    start=True, stop=True,
                )
            p_row = singles.tile([1, IC], F32, tag="p_row")
            nc.vector.tensor_mul(p_row, s_ps, e_row)
            nc.vector.tensor_scalar(
                out=p_row, in0=p_row, scalar1=b_sb[0:1, 0:1], scalar2=None,
                op0=mybir.AluOpType.add,
            )
            nc.sync.dma_start(out=out[:, :], in_=p_row)

    nc.finalize()
    return nc


def kernel(X, X_train, alphas, y_train, b):
    X = np.ascontiguousarray(np.asarray(X, dtype=np.float32))
    X_train = np.ascontiguousarray(np.asarray(X_train, dtype=np.float32))
    alphas = np.asarray(alphas, dtype=np.float32).reshape(M)
    y_train = np.asarray(y_train, dtype=np.float32).reshape(M)
    b_arr = np.asarray(b, dtype=np.float32).reshape(1, 1)

    # Sort train points by label (+1 first) so signs are tile-uniform.
    perm = np.argsort(-y_train, kind="stable")
    n_pos = int((y_train > 0).sum())
    Xt_p = X_train[perm]
    al_p = alphas[perm]

    c = (-GAMMA * (Xt_p * Xt_p).sum(1)
         + np.log(np.maximum(al_p, np.float32(1e-38)))).astype(np.float32)
    cj = np.ascontiguousarray(c.reshape(JT, 128).T)          # [128, JT]
    r = n_pos % 128
    sgn_vec = np.where(np.arange(128) < r, 1.0, -1.0).astype(
        ml_dtypes.bfloat16).reshape(128, 1)
    xt_t = np.ascontiguousarray(Xt_p.T.astype(ml_dtypes.bfloat16))  # [D, M]
    nxsq_full = (-GAMMA * (X * X).sum(1)).astype(np.float32)

    in_maps = []
    for k in range(NCORES):
        sl = slice(k * IC, (k + 1) * IC)
        in_maps.append({
            "xt_t": xt_t,
            "x_t": np.ascontiguousarray(X[sl].T.astype(ml_dtypes.bfloat16)),
            "cj": cj,
            "sgn": sgn_vec,
            "nxsq": np.ascontiguousarray(nxsq_full[sl].reshape(1, IC)),
            "bb": b_arr,
        })

    nc = _build_program(n_pos)
    res = run_bass_kernel_spmd(nc, in_maps, list(range(NCORES)), trace=TRACE)
    global LAST_RESULTS
    LAST_RESULTS = res

    preds = np.concatenate([res.results[k]["out"][0] for k in range(NCORES)])
    return preds.reshape(N, 1).astype(np.float32)

